# revision 60
# baseline (speedup 1.0000x reference)
"""Trainium2 Bass kernel for a local-attention block (MQA, RoPE, causal mask).

Reference computation (B=2, T=2048, WIDTH=2560, 10 q-heads, 1 kv-head,
head_dim=256, window=2048 => mask reduces to causal & same-segment):

    q = x @ wq.T ; k = x @ wk.T ; v = x @ wv.T
    q, k = rope(q), rope(k)
    probs = softmax(q k^T / 16 + mask)
    out = (probs @ v) @ w_final.T + b_final

Sharding: 8 cores = 2 batches x 4 ranks. Load-balanced causal split: rank r
owns the four 128-token query chunks {r, r+4, r+8, r+12}, placed in "slots"
ordered by decreasing causal coverage (16, 12, 8, 4 key tiles of 128). For
key tile t only the first needed(t) = 4 - t//4 slots are processed, so the
S/O matmuls use contiguous column prefixes of the slot-ordered Q buffer and
every core does identical work (SPMD) with no fully-masked tiles computed.

K/V projection is sharded: each core projects+ropes K/V for its own 512
tokens, then an AllGather over the 4 cores of a batch reconstructs the full
K^T / V in every core. The gather runs on the collective cores concurrently
with the Q projection.

Device layouts are "feature on partitions, tokens on free dim" so every
matmul contraction maps to the 128-partition axis with no on-device
transposes (except 128x128 PE transposes of the attention output).
"""

import sys

import numpy as np

for _p in ("/opt/trn_rl_repo", "/root/.axon_site/_ro/trn_rl_repo"):
    if _p not in sys.path:
        sys.path.insert(0, _p)

import ml_dtypes

BF16 = ml_dtypes.bfloat16

B, T, WIDTH = 2, 2048, 2560
NUM_HEADS, HEAD_DIM = 10, 256
WINDOW = 2048
MAX_WAVELENGTH = 10000.0
QBLK = 512              # query tokens per core (4 chunks of 128, slot order)
KVB = 512               # kv tokens projected per core
NW = WIDTH // 128       # 20 width stripes
NTT = T // 128          # 16 key token tiles
VROW = HEAD_DIM + 1     # v columns + ones column (denominator trick)
NSLOT = 4
NCOV = [16, 12, 8, 4]   # key-tile coverage per slot
# key tiles grouped into shared PSUM banks (equal needed() within a group)
TGROUPS = [[0], [1], [2], [3], [4], [5], [6], [7], [8, 9], [10, 11],
           [12, 13, 14, 15]]


def _needed(t):
    return 4 - t // 4


# mask/pt column offset of key tile t
MOFF = [0] * NTT
for _t in range(1, NTT):
    MOFF[_t] = MOFF[_t - 1] + 128 * _needed(_t - 1)
MCOLS = MOFF[-1] + 128 * _needed(NTT - 1)  # 5120

_NC_CACHE = {}


def _build_nc():
    """Build the (single, SPMD-uniform) Bass/Tile program."""
    import concourse.bass as bass  # noqa: F401
    import concourse.mybir as mybir
    import concourse.tile as tile
    from concourse import bacc

    fp32 = mybir.dt.float32
    bf16 = mybir.dt.bfloat16
    Exp = mybir.ActivationFunctionType.Exp
    Copy = mybir.ActivationFunctionType.Copy

    nc = bacc.Bacc("TRN2", target_bir_lowering=False, debug=False, num_devices=8)

    # ---- DRAM I/O ----
    xq = nc.dram_tensor("xq", [NW, 128, QBLK], bf16, kind="ExternalInput")
    # per width-stripe k: [wk_k (256) | wv_k (256) | xkv_k (512)] columns
    kvw_d = nc.dram_tensor("kvw", [128, NW * 1024], bf16, kind="ExternalInput")
    wq = nc.dram_tensor("wq", [NW, 128, WIDTH], bf16, kind="ExternalInput")
    wf = nc.dram_tensor("wf", [NW, 128, WIDTH], bf16, kind="ExternalInput")
    # rope tables packed 2-high: A = [cos; sin], B = [sin; cos] so every
    # DVE multiply sees equal SBUF base partitions
    ka_d = nc.dram_tensor("ka", [128, KVB], bf16, kind="ExternalInput")
    kb_d = nc.dram_tensor("kb", [128, KVB], bf16, kind="ExternalInput")
    qa_d = nc.dram_tensor("qa", [128, QBLK], bf16, kind="ExternalInput")
    qb_d = nc.dram_tensor("qb", [128, QBLK], bf16, kind="ExternalInput")
    msk = nc.dram_tensor("msk", [128, MCOLS], bf16, kind="ExternalInput")
    bia = nc.dram_tensor("bia", [128, NW], fp32, kind="ExternalInput")
    out = nc.dram_tensor("out", [NW, 128, QBLK], fp32, kind="ExternalOutput")

    with tile.TileContext(nc) as tc:
        with (
            tc.tile_pool(name="res", bufs=1) as res,
            tc.tile_pool(name="wstr", bufs=4) as wstr,
            tc.tile_pool(name="ptp", bufs=8) as ptp,
            tc.tile_pool(name="enp", bufs=4) as enp,
            tc.tile_pool(name="tmp", bufs=4) as tmpp,
            tc.tile_pool(name="rcp", bufs=4) as rcpp,
            tc.tile_pool(name="outp", bufs=3) as outp,
            tc.tile_pool(name="dram", bufs=1, space="DRAM") as dram,
            tc.tile_pool(name="pp", bufs=2, space="PSUM") as pp,
            tc.tile_pool(name="stp", bufs=2, space="PSUM") as stp,
            tc.tile_pool(name="op", bufs=4, space="PSUM") as op,
        ):
            # ---- resident SBUF tiles ----
            kvw = res.tile([128, NW * 1024], bf16, tag="kvw")
            xqs = res.tile([128, NW * QBLK], bf16, tag="xqs")
            qtr = res.tile([128, NW * QBLK], bf16, tag="qtr")   # rope'd Q^T
            # gathered K^T rank-major: rank r cols [1024r:1024r+512] = hd
            # half 0 (rope'd), [+512:+1024] = hd half 1, token = 512r + col%512
            ktr01 = res.tile([128, 4096], bf16, tag="ktr01")
            vsb = res.tile([128, NTT * VROW], bf16, tag="vsb")  # V tiles + ones col
            kvsh = res.tile([128, 2048], bf16, tag="kvsh")      # own K/V shard
            enct = res.tile([128, NW * QBLK], bf16, tag="enct")
            ka_s = res.tile([128, KVB], bf16, tag="ka")
            kb_s = res.tile([128, KVB], bf16, tag="kb")
            qa_s = res.tile([128, QBLK], bf16, tag="qa")
            qb_s = res.tile([128, QBLK], bf16, tag="qb")
            msk_s = res.tile([128, MCOLS], bf16, tag="msk")
            bia_s = res.tile([128, NW], fp32, tag="bia")
            kv_in = dram.tile([128, 2048], bf16, tag="kvi")
            kv_out = dram.tile([4, 128, 2048], bf16, tag="kvo")

            # PE p-state warmup: garbage matmuls (results never read) keep
            # the tensor engine continuously busy through its slow-clock ramp
            # while the first input chunks are still in flight.
            for wu in range(13):
                wps = stp.tile([128, QBLK], fp32, tag="st", name=f"wu{wu}")
                fr = QBLK if wu < 7 else 128
                nc.tensor.matmul(
                    wps[:, 0:fr], lhsT=qtr[:, 0:128], rhs=enct[:, 0:fr],
                    start=True, stop=True)

            # ---- input DMAs (SP queue, in consumption order) ----
            # packed [wk|wv|xkv] stripes stream in chunks (small first
            # chunks) so the K/V projection starts early.
            kcs = [0, 1, 2, 3, 5, 7, 9, 11, 14, 17, 20]
            for kc in range(len(kcs) - 1):
                cs = slice(kcs[kc] * 1024, kcs[kc + 1] * 1024)
                nc.sync.dma_start(out=kvw[:, cs], in_=kvw_d[:, cs])
            nc.sync.dma_start(out=ka_s[:], in_=ka_d[:])
            nc.sync.dma_start(out=kb_s[:], in_=kb_d[:])

            wq_tiles = {}

            def load_w(src, m):
                t = wstr.tile([128, WIDTH], bf16, tag="w")
                nc.sync.dma_start(out=t[:], in_=src[m])
                wq_tiles[(src.name, m)] = t

            def load_xq(kc):
                ks = slice(5 * kc, 5 * (kc + 1))
                nc.sync.dma_start(
                    out=xqs[:].rearrange("p (k c) -> p k c", k=NW)[:, ks],
                    in_=xq[:].rearrange("k p c -> p k c")[:, ks])

            load_w(wq, 0)
            load_xq(0)
            load_xq(1)
            load_xq(2)
            load_xq(3)
            load_w(wq, 1)
            load_w(wq, 2)

            # q-rope tables ride the Pool trigger queue: their transfers
            # enter the DMA FIFO before the wq stream jams it, so the later
            # kv_in store (collective critical path) queues earlier too
            nc.gpsimd.dma_start(out=qa_s[:], in_=qa_d[:])
            nc.gpsimd.dma_start(out=qb_s[:], in_=qb_d[:])
            # ones columns of V (denominator of softmax via matmul)
            nc.gpsimd.memset(
                vsb[:].rearrange("p (n v) -> p n v", n=NTT)[:, :, HEAD_DIM:VROW],
                1.0)

            def rope_evict(ps, ta, tb, dst0, dst1):
                """dst0 = ps0*cos - ps1*sin ; dst1 = ps1*cos + ps0*sin.

                ps: [128, n] PSUM fp32; ta/tb: [128, n] SBUF bf16 tables
                (ta = [cos; sin], tb = [sin; cos]); dst0/dst1: bf16 SBUF APs
                [64, n]. The PSUM->bf16 cast runs on Act so every DVE op is
                2-byte (2x mode) with equal SBUF base partitions."""
                n = ta.shape[-1]
                sb = tmpp.tile([128, QBLK], bf16, tag="sb", name="ropesb")
                nc.scalar.copy(out=sb[:, :n], in_=ps[:])
                t0 = tmpp.tile([64, QBLK], bf16, tag="t0", name="t0")
                t1 = tmpp.tile([64, QBLK], bf16, tag="t1", name="t1")
                nc.vector.tensor_mul(t0[:, :n], sb[0:64, :n], ta[0:64, :])
                nc.vector.tensor_mul(t1[:, :n], sb[64:128, :n], ta[64:128, :])
                nc.vector.tensor_sub(dst0, t0[:, :n], t1[:, :n])
                t2 = tmpp.tile([64, QBLK], bf16, tag="t0", name="t2")
                t3 = tmpp.tile([64, QBLK], bf16, tag="t1", name="t3")
                nc.vector.tensor_mul(t2[:, :n], sb[64:128, :n], tb[64:128, :])
                nc.vector.tensor_mul(t3[:, :n], sb[0:64, :n], tb[0:64, :])
                nc.vector.tensor_add(dst1, t2[:, :n], t3[:, :n])

            # ---- K/V shard projection (own 512 tokens) ----
            # kvsh cols: [0:512] rope'd K^T hd0:128, [512:1024] K^T hd128:256,
            # [1024:2048] V tiles (4 x [128tok, 256hd])
            psk0 = pp.tile([128, KVB], fp32, tag="pp", name="psk0")
            psk1 = pp.tile([128, KVB], fp32, tag="pp", name="psk1")
            psv = [op.tile([128, HEAD_DIM], fp32, tag="o", name=f"psv{mt}")
                   for mt in range(4)]
            for k in range(NW):
                wk_c, wv_c, xk_c = 1024 * k, 1024 * k + 256, 1024 * k + 512
                nc.tensor.matmul(
                    psk0[:], lhsT=kvw[:, wk_c:wk_c + 128],
                    rhs=kvw[:, xk_c:xk_c + KVB],
                    start=(k == 0), stop=(k == NW - 1))
                nc.tensor.matmul(
                    psk1[:], lhsT=kvw[:, wk_c + 128:wk_c + 256],
                    rhs=kvw[:, xk_c:xk_c + KVB],
                    start=(k == 0), stop=(k == NW - 1))
                for mt in range(4):
                    nc.tensor.matmul(
                        psv[mt][:],
                        lhsT=kvw[:, xk_c + mt * 128:xk_c + (mt + 1) * 128],
                        rhs=kvw[:, wv_c:wv_c + 256],
                        start=(k == 0), stop=(k == NW - 1))
            rope_evict(psk0, ka_s[:], kb_s[:],
                       kvsh[0:64, 0:KVB], kvsh[64:128, 0:KVB])
            nc.scalar.copy(out=kvsh[:, KVB:2 * KVB], in_=psk1[:])
            for mt in range(4):
                nc.scalar.copy(
                    out=kvsh[:, 1024 + mt * HEAD_DIM: 1024 + (mt + 1) * HEAD_DIM],
                    in_=psv[mt][:])

            # ---- K/V all-gather across the 4 cores of this batch ----
            nc.gpsimd.dma_start(out=kv_in[:], in_=kvsh[:])
            nc.gpsimd.collective_compute(
                "AllGather",
                mybir.AluOpType.bypass,
                replica_groups=[[0, 1, 2, 3], [4, 5, 6, 7]],
                ins=[kv_in.opt()],
                outs=[kv_out.opt()],
            )
            # ---- Q projection -> rope'd Q^T stripes [qdim, QBLK] ----
            # stripe m: qdim rows [128m, 128m+128) = head m//2, half m%2
            for m in range(NW):
                if 2 <= m and m + 1 < NW:
                    load_w(wq, m + 1)
                wq_m = wq_tiles.pop(("wq", m))
                ps = pp.tile([128, QBLK], fp32, tag="pp")
                for k in range(NW):
                    nc.tensor.matmul(
                        ps[:],
                        lhsT=wq_m[:, k * 128:(k + 1) * 128],
                        rhs=xqs[:, k * QBLK:(k + 1) * QBLK],
                        start=(k == 0),
                        stop=(k == NW - 1),
                    )
                dst = qtr[:, m * QBLK:(m + 1) * QBLK]
                if m % 2 == 0:  # rope half of the head dims
                    rope_evict(ps, qa_s[:], qb_s[:],
                               qtr[0:64, m * QBLK:(m + 1) * QBLK],
                               qtr[64:128, m * QBLK:(m + 1) * QBLK])
                else:           # passthrough half
                    nc.scalar.copy(out=dst, in_=ps[:])

            # masks + bias arrive behind the wq stripes, before attention
            nc.sync.dma_start(out=msk_s[:], in_=msk[:])
            nc.sync.dma_start(out=bia_s[:], in_=bia[:])

            # post-collective loads stay on the Pool queue: any other
            # engine's queue would head-of-line-block later triggers behind
            # the collective-semaphore wait
            nc.gpsimd.dma_start(
                out=ktr01[:].rearrange("p (r c) -> p r c", r=4),
                in_=kv_out[:, :, 0:1024].rearrange("r p c -> p r c"))
            for r4 in range(4):
                nc.gpsimd.dma_start(
                    out=vsb[:].rearrange("p (n v) -> p n v", n=NTT)[
                        :, 4 * r4:4 * (r4 + 1), 0:HEAD_DIM],
                    in_=kv_out[r4, :, 1024:2048].rearrange(
                        "p (t v) -> p t v", t=4))


            # ---- attention (S^T layout: k on partitions, q on free dim) ----
            def evict_slot(h, p, o_tile):
                r = rcpp.tile([128, 1], fp32, tag="r")
                nc.vector.reciprocal(r[:], o_tile[:, HEAD_DIM:VROW])
                en = enp.tile([128, HEAD_DIM], bf16, tag="en")
                nc.scalar.activation(en[:], o_tile[:, 0:HEAD_DIM], Copy,
                                     scale=r[:])
                for hh in range(2):
                    tp = pp.tile([128, 128], bf16, tag="pp")
                    nc.tensor.matmul(
                        tp[:], lhsT=en[:, hh * 128:(hh + 1) * 128],
                        rhs=ident[:], is_transpose=True)
                    nc.vector.tensor_copy(
                        enct[:, (2 * h + hh) * QBLK + p * 128:
                             (2 * h + hh) * QBLK + (p + 1) * 128],
                        tp[:])

            # Software-pipelined over a flat (head, group) stream: the O
            # matmuls lag one group behind S/exp/mask so the Act+DVE latency
            # between S and O is never exposed on the tensor engine; the
            # eviction transposes lag one more group.
            all_groups = [(h, grp) for h in range(NUM_HEADS) for grp in TGROUPS]
            o_by_head = {}
            pts = {}

            def emit_s(i):
                h, grp = all_groups[i]
                if grp is TGROUPS[0]:
                    o_by_head[h] = [
                        op.tile([128, VROW], fp32, tag="o", name=f"o{h}_{p}")
                        for p in range(NSLOT)]
                nd = _needed(grp[0])
                gw = 128 * nd * len(grp)   # group column width
                # alternate PSUM pools: pp is otherwise idle during
                # attention, doubling the S-tile buffering depth
                st = (stp if i % 2 == 0 else pp).tile(
                    [128, QBLK], fp32, tag="st" if i % 2 == 0 else "pp")
                for j, t in enumerate(grp):
                    cols = slice(j * 128 * nd, (j + 1) * 128 * nd)
                    koff = 1024 * (t // 4) + 128 * (t % 4)
                    nc.tensor.matmul(
                        st[:, cols], lhsT=ktr01[:, koff:koff + 128],
                        rhs=qtr[:, (2 * h) * QBLK:(2 * h) * QBLK + 128 * nd],
                        start=True, stop=False)
                    nc.tensor.matmul(
                        st[:, cols], lhsT=ktr01[:, koff + 512:koff + 640],
                        rhs=qtr[:, (2 * h + 1) * QBLK:
                                 (2 * h + 1) * QBLK + 128 * nd],
                        start=False, stop=True)
                pt = ptp.tile([128, QBLK], bf16, tag="pt")
                # p = exp(s / sqrt(head_dim)), masked entries -> 0
                nc.scalar.activation(pt[:, :gw], st[:, :gw], Exp, scale=0.0625)
                pts[i] = pt

            def emit_mask(i):
                h, grp = all_groups[i]
                gw = 128 * _needed(grp[0]) * len(grp)
                pt = pts[i]
                nc.vector.tensor_mul(
                    pt[:, :gw], pt[:, :gw],
                    msk_s[:, MOFF[grp[0]]:MOFF[grp[0]] + gw])

            deferred_p0 = {}

            def emit_o(i):
                h, grp = all_groups[i]
                nd = _needed(grp[0])
                pt = pts.pop(i)
                for j, t in enumerate(grp):
                    for p in reversed(range(nd)):
                        lp = pt[:, j * 128 * nd + p * 128:
                                j * 128 * nd + (p + 1) * 128]
                        if p == 0 and t == 0 and h > 0:
                            # defer slot 0's t=0 term one group: its PSUM
                            # slot is the last one the previous head frees
                            deferred_p0[h] = (pt, lp)
                            continue
                        if p == 0 and t == 1 and h in deferred_p0:
                            nc.tensor.matmul(
                                o_by_head[h][0][:], lhsT=lp,
                                rhs=vsb[:, VROW:2 * VROW],
                                start=True, stop=False)
                            _, lp0 = deferred_p0.pop(h)
                            nc.tensor.matmul(
                                o_by_head[h][0][:], lhsT=lp0,
                                rhs=vsb[:, 0:VROW],
                                start=False, stop=False)
                            continue
                        nc.tensor.matmul(
                            o_by_head[h][p][:],
                            lhsT=lp,
                            rhs=vsb[:, t * VROW:(t + 1) * VROW],
                            start=(t == 0),
                            stop=(t == NCOV[p] - 1),
                        )

            evq = []   # (h, p, en) awaiting their PE transposes

            def emit_evict_scale(i):
                h, grp = all_groups[i]
                for p in range(NSLOT):
                    if NCOV[p] - 1 != grp[-1]:
                        continue
                    o_tile = o_by_head[h][p]
                    r = rcpp.tile([128, 1], fp32, tag="r")
                    nc.vector.reciprocal(r[:], o_tile[:, HEAD_DIM:VROW])
                    en = enp.tile([128, HEAD_DIM], bf16, tag="en")
                    nc.scalar.activation(en[:], o_tile[:, 0:HEAD_DIM], Copy,
                                         scale=r[:])
                    evq.append((h, p, en))

            def emit_transposes():
                # XBAR DMA transposes: keeps the tensor engine and DVE out
                # of the eviction path entirely (SP + DMA are idle here).
                while evq:
                    h, p, en = evq.pop(0)
                    for hh in range(2):
                        nc.sync.dma_start_transpose(
                            out=enct[:, (2 * h + hh) * QBLK + p * 128:
                                     (2 * h + hh) * QBLK + (p + 1) * 128],
                            in_=en[:, hh * 128:(hh + 1) * 128])

            # wf prefetch: triggers fire on the idle SP queue during attention
            load_w(wf, 0)
            load_w(wf, 1)

            NG = len(all_groups)
            LAG = 3
            for i in range(NG):
                emit_s(i)
                if i >= LAG:
                    emit_o(i - LAG)
                    emit_evict_scale(i - LAG)   # recip/scale queue ahead...
                emit_mask(i)                    # ...of this group's mask-mul
                if i >= LAG:
                    emit_transposes()       # drain earlier evictions

            # Attention tail interleaved with the first final-proj stripes:
            # heads 0..8 (k=0..17) of stripes 0/1 accumulate while head 9's
            # last O/eviction chains drain, hiding their latency.
            fin_ps = {}

            def final_partial(m, kr):
                if m not in fin_ps:
                    fin_ps[m] = stp.tile([128, QBLK], fp32, tag="st",
                                         name=f"fps{m}")
                for k in kr:
                    nc.tensor.matmul(
                        fin_ps[m][:],
                        lhsT=wq_tiles[("wf", m)][:, k * 128:(k + 1) * 128],
                        rhs=enct[:, k * QBLK:(k + 1) * QBLK],
                        start=(k == 0),
                        stop=(k == NW - 1),
                    )

            emit_o(NG - 3)
            emit_evict_scale(NG - 3)
            emit_o(NG - 2)
            emit_evict_scale(NG - 2)
            final_partial(0, range(0, 18))
            emit_o(NG - 1)
            emit_evict_scale(NG - 1)
            final_partial(1, range(0, 18))
            emit_transposes()

            # ---- final projection: out^T = wf @ enc^T + bias ----
            for m in range(NW):
                if m + 2 < NW:
                    load_w(wf, m + 2)
                wf_m = wq_tiles.pop(("wf", m))
                if m in fin_ps:  # heads 0..8 already accumulated above
                    ps = fin_ps.pop(m)
                    for k in range(18, NW):
                        nc.tensor.matmul(
                            ps[:],
                            lhsT=wf_m[:, k * 128:(k + 1) * 128],
                            rhs=enct[:, k * QBLK:(k + 1) * QBLK],
                            start=(k == 0),
                            stop=(k == NW - 1),
                        )
                    osb = outp.tile([128, QBLK], fp32, tag="osb")
                    nc.vector.tensor_scalar_add(osb[:], ps[:], bia_s[:, m:m + 1])
                    nc.sync.dma_start(out=out[m], in_=osb[:])
                    continue
                ps = pp.tile([128, QBLK], fp32, tag="pp")
                if m < NW - 1:
                    for k in range(NW):
                        nc.tensor.matmul(
                            ps[:],
                            lhsT=wf_m[:, k * 128:(k + 1) * 128],
                            rhs=enct[:, k * QBLK:(k + 1) * QBLK],
                            start=(k == 0),
                            stop=(k == NW - 1),
                        )
                    osb = outp.tile([128, QBLK], fp32, tag="osb")
                    nc.vector.tensor_scalar_add(osb[:], ps[:], bia_s[:, m:m + 1])
                    nc.sync.dma_start(out=out[m], in_=osb[:])
                else:
                    # last stripe in column halves (separate PSUM tiles): the
                    # first half's bias/store drains under the second half
                    osb = outp.tile([128, QBLK], fp32, tag="osb")
                    pieces = [(0, 256), (256, 384), (384, 512)]
                    for j2, (c0, c1) in enumerate(pieces):
                        cw = c1 - c0
                        psj = ps if j2 == 0 else pp.tile(
                            [128, QBLK], fp32, tag="pp")
                        for k in range(NW):
                            nc.tensor.matmul(
                                psj[:, 0:cw],
                                lhsT=wf_m[:, k * 128:(k + 1) * 128],
                                rhs=enct[:, k * QBLK + c0:k * QBLK + c1],
                                start=(k == 0),
                                stop=(k == NW - 1),
                            )
                        nc.vector.tensor_scalar_add(
                            osb[:, c0:c1], psj[:, 0:cw], bia_s[:, m:m + 1])
                        nc.sync.dma_start(
                            out=out[m][:, c0:c1], in_=osb[:, c0:c1])

    if not nc.is_finalized():
        nc.finalize()  # bacc register allocation — required before walrus compile
    return nc


def get_nc():
    if "nc" not in _NC_CACHE:
        _NC_CACHE["nc"] = _build_nc()
    return _NC_CACHE["nc"]


def _chunk_of_slot(r, p):
    """Physical 128-token query chunk held by rank r's slot p."""
    return r + 12 - 4 * p


def _host_prepare(x, segment_pos, wq, wk, wv, w_final, b_final):
    """Build shared + per-core device input arrays."""
    x = np.asarray(x, dtype=np.float32)
    segment_pos = np.asarray(segment_pos)
    wq = np.asarray(wq, dtype=np.float32)
    wk = np.asarray(wk, dtype=np.float32)
    wv = np.asarray(wv, dtype=np.float32)
    w_final = np.asarray(w_final, dtype=np.float32)
    b_final = np.asarray(b_final, dtype=np.float32)

    def stripes_sq(w):  # [WIDTH, WIDTH] torch-Linear weight -> [20,128,WIDTH] w^T stripes
        wt = np.ascontiguousarray(w.T)
        return np.ascontiguousarray(
            wt.reshape(NW, 128, NW, 128).transpose(2, 1, 0, 3).reshape(NW, 128, WIDTH)
        ).astype(BF16)

    def packed_kv(w):  # [HEAD_DIM, WIDTH] -> [128, NW, HEAD_DIM] w^T stripes
        return w.T.reshape(NW, 128, HEAD_DIM).transpose(1, 0, 2)

    wk_p = packed_kv(wk)
    wv_p = packed_kv(wv)

    shared = {
        "wq": stripes_sq(wq),
        "wf": stripes_sq(w_final),
        "bia": np.ascontiguousarray(b_final.reshape(NW, 128).T).astype(np.float32),
    }

    inv_freq = (
        1.0 / MAX_WAVELENGTH ** (2.0 * np.arange(HEAD_DIM // 4, dtype=np.float32)
                                 / (HEAD_DIM // 2))
    ).astype(np.float32)

    def cossin(pos):
        ang = inv_freq[:, None] * pos[None, :].astype(np.float32)
        return (np.cos(ang).astype(BF16), np.sin(ang).astype(BF16))

    in_maps = []
    for c in range(8):
        b, r = c // 4, c % 4
        pos = segment_pos[b].astype(np.float32)
        seg = np.cumsum((segment_pos[b] == 0).astype(np.int64))

        qidx = np.concatenate(
            [np.arange(128) + 128 * _chunk_of_slot(r, p) for p in range(NSLOT)])
        kidx = np.arange(KVB) + KVB * r

        xqc = np.ascontiguousarray(x[b][qidx].T).astype(BF16).reshape(NW, 128, QBLK)
        xkv_p = x[b][kidx].T.reshape(NW, 128, KVB).transpose(1, 0, 2)
        kvw_c = np.empty((128, NW, 1024), dtype=BF16)
        kvw_c[:, :, 0:256] = wk_p
        kvw_c[:, :, 256:512] = wv_p
        kvw_c[:, :, 512:1024] = xkv_p
        kvw_c = kvw_c.reshape(128, NW * 1024)

        csq_, snq_ = cossin(pos[qidx])
        csk_, snk_ = cossin(pos[kidx])
        qa_ = np.vstack([csq_, snq_])
        qb_ = np.vstack([snq_, csq_])
        ka_ = np.vstack([csk_, snk_])
        kb_ = np.vstack([snk_, csk_])

        # packed masks: for key tile t, needed(t) slot blocks of [128k,128q]
        mask = np.zeros((128, MCOLS), dtype=BF16)
        for t in range(NTT):
            tk = np.arange(128) + 128 * t
            for p in range(_needed(t)):
                tq = qidx[p * 128:(p + 1) * 128]
                allow = (
                    (tk[:, None] <= tq[None, :])
                    & (tq[None, :] <= tk[:, None] + WINDOW)
                    & (seg[tk][:, None] == seg[tq][None, :])
                )
                mask[:, MOFF[t] + 128 * p: MOFF[t] + 128 * (p + 1)] = allow
        in_maps.append(dict(
            shared,
            xq=xqc,
            kvw=kvw_c,
            ka=ka_,
            kb=kb_,
            qa=qa_,
            qb=qb_,
            msk=mask,
        ))
    return in_maps


def _assemble(results):
    out = np.empty((B, T, WIDTH), dtype=np.float32)
    for c, res in enumerate(results):
        b, r = c // 4, c % 4
        o = np.asarray(res["out"], dtype=np.float32)  # [NW, 128, QBLK]
        for p in range(NSLOT):
            ch = _chunk_of_slot(r, p)
            out[b, 128 * ch:128 * (ch + 1), :] = (
                o[:, :, 128 * p:128 * (p + 1)].transpose(2, 0, 1).reshape(128, WIDTH)
            )
    return out


def kernel(x, segment_pos, wq, wk, wv, w_final, b_final):
    from concourse.bass_utils import run_bass_kernel_spmd

    nc = get_nc()
    in_maps = _host_prepare(x, segment_pos, wq, wk, wv, w_final, b_final)
    res = run_bass_kernel_spmd(nc, in_maps, list(range(8)))
    return _assemble(res.results)


# revision 61
# speedup vs baseline: 1.0053x; 1.0053x over previous
"""Trainium2 Bass kernel for a local-attention block (MQA, RoPE, causal mask).

Reference computation (B=2, T=2048, WIDTH=2560, 10 q-heads, 1 kv-head,
head_dim=256, window=2048 => mask reduces to causal & same-segment):

    q = x @ wq.T ; k = x @ wk.T ; v = x @ wv.T
    q, k = rope(q), rope(k)
    probs = softmax(q k^T / 16 + mask)
    out = (probs @ v) @ w_final.T + b_final

Sharding: 8 cores = 2 batches x 4 ranks. Load-balanced causal split: rank r
owns the four 128-token query chunks {r, r+4, r+8, r+12}, placed in "slots"
ordered by decreasing causal coverage (16, 12, 8, 4 key tiles of 128). For
key tile t only the first needed(t) = 4 - t//4 slots are processed, so the
S/O matmuls use contiguous column prefixes of the slot-ordered Q buffer and
every core does identical work (SPMD) with no fully-masked tiles computed.

K/V projection is sharded: each core projects+ropes K/V for its own 512
tokens, then an AllGather over the 4 cores of a batch reconstructs the full
K^T / V in every core. The gather runs on the collective cores concurrently
with the Q projection.

Device layouts are "feature on partitions, tokens on free dim" so every
matmul contraction maps to the 128-partition axis with no on-device
transposes (except 128x128 PE transposes of the attention output).
"""

import sys

import numpy as np

for _p in ("/opt/trn_rl_repo", "/root/.axon_site/_ro/trn_rl_repo"):
    if _p not in sys.path:
        sys.path.insert(0, _p)

import ml_dtypes

BF16 = ml_dtypes.bfloat16

B, T, WIDTH = 2, 2048, 2560
NUM_HEADS, HEAD_DIM = 10, 256
WINDOW = 2048
MAX_WAVELENGTH = 10000.0
QBLK = 512              # query tokens per core (4 chunks of 128, slot order)
KVB = 512               # kv tokens projected per core
NW = WIDTH // 128       # 20 width stripes
NTT = T // 128          # 16 key token tiles
VROW = HEAD_DIM + 1     # v columns + ones column (denominator trick)
NSLOT = 4
NCOV = [16, 12, 8, 4]   # key-tile coverage per slot
# key tiles grouped into shared PSUM banks (equal needed() within a group)
TGROUPS = [[0], [1], [2], [3], [4], [5], [6], [7], [8, 9], [10, 11],
           [12, 13, 14, 15]]


def _needed(t):
    return 4 - t // 4


# mask/pt column offset of key tile t
MOFF = [0] * NTT
for _t in range(1, NTT):
    MOFF[_t] = MOFF[_t - 1] + 128 * _needed(_t - 1)
MCOLS = MOFF[-1] + 128 * _needed(NTT - 1)  # 5120

_NC_CACHE = {}


def _build_nc():
    """Build the (single, SPMD-uniform) Bass/Tile program."""
    import concourse.bass as bass  # noqa: F401
    import concourse.mybir as mybir
    import concourse.tile as tile
    from concourse import bacc

    fp32 = mybir.dt.float32
    bf16 = mybir.dt.bfloat16
    Exp = mybir.ActivationFunctionType.Exp
    Copy = mybir.ActivationFunctionType.Copy

    nc = bacc.Bacc("TRN2", target_bir_lowering=False, debug=False, num_devices=8)

    # ---- DRAM I/O ----
    xq = nc.dram_tensor("xq", [NW, 128, QBLK], bf16, kind="ExternalInput")
    # per width-stripe k: [wk_k (256) | wv_k (256) | xkv_k (512)] columns
    kvw_d = nc.dram_tensor("kvw", [128, NW * 1024], bf16, kind="ExternalInput")
    wq = nc.dram_tensor("wq", [NW, 128, WIDTH], bf16, kind="ExternalInput")
    wf = nc.dram_tensor("wf", [NW, 128, WIDTH], bf16, kind="ExternalInput")
    # rope tables packed 2-high: A = [cos; sin], B = [sin; cos] so every
    # DVE multiply sees equal SBUF base partitions
    ka_d = nc.dram_tensor("ka", [128, KVB], bf16, kind="ExternalInput")
    kb_d = nc.dram_tensor("kb", [128, KVB], bf16, kind="ExternalInput")
    qa_d = nc.dram_tensor("qa", [128, QBLK], bf16, kind="ExternalInput")
    qb_d = nc.dram_tensor("qb", [128, QBLK], bf16, kind="ExternalInput")
    msk = nc.dram_tensor("msk", [128, MCOLS], bf16, kind="ExternalInput")
    bia = nc.dram_tensor("bia", [128, NW], fp32, kind="ExternalInput")
    out = nc.dram_tensor("out", [NW, 128, QBLK], fp32, kind="ExternalOutput")

    with tile.TileContext(nc) as tc:
        with (
            tc.tile_pool(name="res", bufs=1) as res,
            tc.tile_pool(name="wstr", bufs=4) as wstr,
            tc.tile_pool(name="ptp", bufs=8) as ptp,
            tc.tile_pool(name="enp", bufs=4) as enp,
            tc.tile_pool(name="tmp", bufs=4) as tmpp,
            tc.tile_pool(name="rcp", bufs=4) as rcpp,
            tc.tile_pool(name="outp", bufs=3) as outp,
            tc.tile_pool(name="dram", bufs=1, space="DRAM") as dram,
            tc.tile_pool(name="pp", bufs=2, space="PSUM") as pp,
            tc.tile_pool(name="stp", bufs=2, space="PSUM") as stp,
            tc.tile_pool(name="op", bufs=4, space="PSUM") as op,
        ):
            # ---- resident SBUF tiles ----
            kvw = res.tile([128, NW * 1024], bf16, tag="kvw")
            xqs = res.tile([128, NW * QBLK], bf16, tag="xqs")
            qtr = res.tile([128, NW * QBLK], bf16, tag="qtr")   # rope'd Q^T
            # gathered K^T rank-major: rank r cols [1024r:1024r+512] = hd
            # half 0 (rope'd), [+512:+1024] = hd half 1, token = 512r + col%512
            ktr01 = res.tile([128, 4096], bf16, tag="ktr01")
            vsb = res.tile([128, NTT * VROW], bf16, tag="vsb")  # V tiles + ones col
            kvsh = res.tile([128, 2048], bf16, tag="kvsh")      # own K/V shard
            enct = res.tile([128, NW * QBLK], bf16, tag="enct")
            ka_s = res.tile([128, KVB], bf16, tag="ka")
            kb_s = res.tile([128, KVB], bf16, tag="kb")
            qa_s = res.tile([128, QBLK], bf16, tag="qa")
            qb_s = res.tile([128, QBLK], bf16, tag="qb")
            msk_s = res.tile([128, MCOLS], bf16, tag="msk")
            bia_s = res.tile([128, NW], fp32, tag="bia")
            kv_in = dram.tile([128, 2048], bf16, tag="kvi")
            kv_out = dram.tile([4, 128, 2048], bf16, tag="kvo")

            # PE p-state warmup: garbage matmuls (results never read) keep
            # the tensor engine continuously busy through its slow-clock ramp
            # while the first input chunks are still in flight.
            for wu in range(13):
                wps = stp.tile([128, QBLK], fp32, tag="st", name=f"wu{wu}")
                fr = QBLK if wu < 7 else 128
                nc.tensor.matmul(
                    wps[:, 0:fr], lhsT=qtr[:, 0:128], rhs=enct[:, 0:fr],
                    start=True, stop=True)

            # ---- input DMAs (SP queue, in consumption order) ----
            # packed [wk|wv|xkv] stripes stream in chunks (small first
            # chunks) so the K/V projection starts early.
            kcs = [0, 1, 2, 3, 5, 7, 9, 11, 14, 17, 20]
            for kc in range(len(kcs) - 1):
                cs = slice(kcs[kc] * 1024, kcs[kc + 1] * 1024)
                nc.sync.dma_start(out=kvw[:, cs], in_=kvw_d[:, cs])
            nc.sync.dma_start(out=ka_s[:], in_=ka_d[:])
            nc.sync.dma_start(out=kb_s[:], in_=kb_d[:])

            wq_tiles = {}

            def load_w(src, m):
                t = wstr.tile([128, WIDTH], bf16, tag="w")
                nc.sync.dma_start(out=t[:], in_=src[m])
                wq_tiles[(src.name, m)] = t

            def load_xq(kc):
                ks = slice(5 * kc, 5 * (kc + 1))
                nc.sync.dma_start(
                    out=xqs[:].rearrange("p (k c) -> p k c", k=NW)[:, ks],
                    in_=xq[:].rearrange("k p c -> p k c")[:, ks])

            load_w(wq, 0)
            load_xq(0)
            load_xq(1)
            load_w(wq, 1)
            load_xq(2)
            load_xq(3)
            load_w(wq, 2)

            # q-rope tables ride the Pool trigger queue: their transfers
            # enter the DMA FIFO before the wq stream jams it, so the later
            # kv_in store (collective critical path) queues earlier too
            nc.gpsimd.dma_start(out=qa_s[:], in_=qa_d[:])
            nc.gpsimd.dma_start(out=qb_s[:], in_=qb_d[:])
            # ones columns of V (denominator of softmax via matmul)
            nc.gpsimd.memset(
                vsb[:].rearrange("p (n v) -> p n v", n=NTT)[:, :, HEAD_DIM:VROW],
                1.0)

            def rope_evict(ps, ta, tb, dst0, dst1):
                """dst0 = ps0*cos - ps1*sin ; dst1 = ps1*cos + ps0*sin.

                ps: [128, n] PSUM fp32; ta/tb: [128, n] SBUF bf16 tables
                (ta = [cos; sin], tb = [sin; cos]); dst0/dst1: bf16 SBUF APs
                [64, n]. The PSUM->bf16 cast runs on Act so every DVE op is
                2-byte (2x mode) with equal SBUF base partitions."""
                n = ta.shape[-1]
                sb = tmpp.tile([128, QBLK], bf16, tag="sb", name="ropesb")
                nc.scalar.copy(out=sb[:, :n], in_=ps[:])
                t0 = tmpp.tile([64, QBLK], bf16, tag="t0", name="t0")
                t1 = tmpp.tile([64, QBLK], bf16, tag="t1", name="t1")
                nc.vector.tensor_mul(t0[:, :n], sb[0:64, :n], ta[0:64, :])
                nc.vector.tensor_mul(t1[:, :n], sb[64:128, :n], ta[64:128, :])
                nc.vector.tensor_sub(dst0, t0[:, :n], t1[:, :n])
                t2 = tmpp.tile([64, QBLK], bf16, tag="t0", name="t2")
                t3 = tmpp.tile([64, QBLK], bf16, tag="t1", name="t3")
                nc.vector.tensor_mul(t2[:, :n], sb[64:128, :n], tb[64:128, :])
                nc.vector.tensor_mul(t3[:, :n], sb[0:64, :n], tb[0:64, :])
                nc.vector.tensor_add(dst1, t2[:, :n], t3[:, :n])

            # ---- K/V shard projection (own 512 tokens) ----
            # kvsh cols: [0:512] rope'd K^T hd0:128, [512:1024] K^T hd128:256,
            # [1024:2048] V tiles (4 x [128tok, 256hd])
            psk0 = pp.tile([128, KVB], fp32, tag="pp", name="psk0")
            psk1 = pp.tile([128, KVB], fp32, tag="pp", name="psk1")
            psv = [op.tile([128, HEAD_DIM], fp32, tag="o", name=f"psv{mt}")
                   for mt in range(4)]
            for k in range(NW):
                wk_c, wv_c, xk_c = 1024 * k, 1024 * k + 256, 1024 * k + 512
                nc.tensor.matmul(
                    psk0[:], lhsT=kvw[:, wk_c:wk_c + 128],
                    rhs=kvw[:, xk_c:xk_c + KVB],
                    start=(k == 0), stop=(k == NW - 1))
                nc.tensor.matmul(
                    psk1[:], lhsT=kvw[:, wk_c + 128:wk_c + 256],
                    rhs=kvw[:, xk_c:xk_c + KVB],
                    start=(k == 0), stop=(k == NW - 1))
                for mt in range(4):
                    nc.tensor.matmul(
                        psv[mt][:],
                        lhsT=kvw[:, xk_c + mt * 128:xk_c + (mt + 1) * 128],
                        rhs=kvw[:, wv_c:wv_c + 256],
                        start=(k == 0), stop=(k == NW - 1))
            rope_evict(psk0, ka_s[:], kb_s[:],
                       kvsh[0:64, 0:KVB], kvsh[64:128, 0:KVB])
            nc.scalar.copy(out=kvsh[:, KVB:2 * KVB], in_=psk1[:])
            for mt in range(4):
                nc.scalar.copy(
                    out=kvsh[:, 1024 + mt * HEAD_DIM: 1024 + (mt + 1) * HEAD_DIM],
                    in_=psv[mt][:])

            # ---- K/V all-gather across the 4 cores of this batch ----
            nc.gpsimd.dma_start(out=kv_in[:], in_=kvsh[:])
            nc.gpsimd.collective_compute(
                "AllGather",
                mybir.AluOpType.bypass,
                replica_groups=[[0, 1, 2, 3], [4, 5, 6, 7]],
                ins=[kv_in.opt()],
                outs=[kv_out.opt()],
            )
            # ---- Q projection -> rope'd Q^T stripes [qdim, QBLK] ----
            # stripe m: qdim rows [128m, 128m+128) = head m//2, half m%2
            for m in range(NW):
                if 2 <= m and m + 1 < NW:
                    load_w(wq, m + 1)
                wq_m = wq_tiles.pop(("wq", m))
                ps = pp.tile([128, QBLK], fp32, tag="pp")
                for k in range(NW):
                    nc.tensor.matmul(
                        ps[:],
                        lhsT=wq_m[:, k * 128:(k + 1) * 128],
                        rhs=xqs[:, k * QBLK:(k + 1) * QBLK],
                        start=(k == 0),
                        stop=(k == NW - 1),
                    )
                dst = qtr[:, m * QBLK:(m + 1) * QBLK]
                if m % 2 == 0:  # rope half of the head dims
                    rope_evict(ps, qa_s[:], qb_s[:],
                               qtr[0:64, m * QBLK:(m + 1) * QBLK],
                               qtr[64:128, m * QBLK:(m + 1) * QBLK])
                else:           # passthrough half
                    nc.scalar.copy(out=dst, in_=ps[:])

            # masks + bias arrive behind the wq stripes, before attention
            nc.sync.dma_start(out=msk_s[:], in_=msk[:])
            nc.sync.dma_start(out=bia_s[:], in_=bia[:])

            # post-collective loads stay on the Pool queue: any other
            # engine's queue would head-of-line-block later triggers behind
            # the collective-semaphore wait
            nc.gpsimd.dma_start(
                out=ktr01[:].rearrange("p (r c) -> p r c", r=4),
                in_=kv_out[:, :, 0:1024].rearrange("r p c -> p r c"))
            for r4 in range(4):
                nc.gpsimd.dma_start(
                    out=vsb[:].rearrange("p (n v) -> p n v", n=NTT)[
                        :, 4 * r4:4 * (r4 + 1), 0:HEAD_DIM],
                    in_=kv_out[r4, :, 1024:2048].rearrange(
                        "p (t v) -> p t v", t=4))


            # ---- attention (S^T layout: k on partitions, q on free dim) ----
            def evict_slot(h, p, o_tile):
                r = rcpp.tile([128, 1], fp32, tag="r")
                nc.vector.reciprocal(r[:], o_tile[:, HEAD_DIM:VROW])
                en = enp.tile([128, HEAD_DIM], bf16, tag="en")
                nc.scalar.activation(en[:], o_tile[:, 0:HEAD_DIM], Copy,
                                     scale=r[:])
                for hh in range(2):
                    tp = pp.tile([128, 128], bf16, tag="pp")
                    nc.tensor.matmul(
                        tp[:], lhsT=en[:, hh * 128:(hh + 1) * 128],
                        rhs=ident[:], is_transpose=True)
                    nc.vector.tensor_copy(
                        enct[:, (2 * h + hh) * QBLK + p * 128:
                             (2 * h + hh) * QBLK + (p + 1) * 128],
                        tp[:])

            # Software-pipelined over a flat (head, group) stream: the O
            # matmuls lag one group behind S/exp/mask so the Act+DVE latency
            # between S and O is never exposed on the tensor engine; the
            # eviction transposes lag one more group.
            all_groups = [(h, grp) for h in range(NUM_HEADS) for grp in TGROUPS]
            o_by_head = {}
            pts = {}

            def emit_s(i):
                h, grp = all_groups[i]
                if grp is TGROUPS[0]:
                    o_by_head[h] = [
                        op.tile([128, VROW], fp32, tag="o", name=f"o{h}_{p}")
                        for p in range(NSLOT)]
                nd = _needed(grp[0])
                gw = 128 * nd * len(grp)   # group column width
                # alternate PSUM pools: pp is otherwise idle during
                # attention, doubling the S-tile buffering depth
                st = (stp if i % 2 == 0 else pp).tile(
                    [128, QBLK], fp32, tag="st" if i % 2 == 0 else "pp")
                for j, t in enumerate(grp):
                    cols = slice(j * 128 * nd, (j + 1) * 128 * nd)
                    koff = 1024 * (t // 4) + 128 * (t % 4)
                    nc.tensor.matmul(
                        st[:, cols], lhsT=ktr01[:, koff:koff + 128],
                        rhs=qtr[:, (2 * h) * QBLK:(2 * h) * QBLK + 128 * nd],
                        start=True, stop=False)
                    nc.tensor.matmul(
                        st[:, cols], lhsT=ktr01[:, koff + 512:koff + 640],
                        rhs=qtr[:, (2 * h + 1) * QBLK:
                                 (2 * h + 1) * QBLK + 128 * nd],
                        start=False, stop=True)
                pt = ptp.tile([128, QBLK], bf16, tag="pt")
                # p = exp(s / sqrt(head_dim)), masked entries -> 0
                nc.scalar.activation(pt[:, :gw], st[:, :gw], Exp, scale=0.0625)
                pts[i] = pt

            def emit_mask(i):
                h, grp = all_groups[i]
                gw = 128 * _needed(grp[0]) * len(grp)
                pt = pts[i]
                nc.vector.tensor_mul(
                    pt[:, :gw], pt[:, :gw],
                    msk_s[:, MOFF[grp[0]]:MOFF[grp[0]] + gw])

            deferred_p0 = {}

            def emit_o(i):
                h, grp = all_groups[i]
                nd = _needed(grp[0])
                pt = pts.pop(i)
                for j, t in enumerate(grp):
                    for p in reversed(range(nd)):
                        lp = pt[:, j * 128 * nd + p * 128:
                                j * 128 * nd + (p + 1) * 128]
                        if p == 0 and t == 0 and h > 0:
                            # defer slot 0's t=0 term one group: its PSUM
                            # slot is the last one the previous head frees
                            deferred_p0[h] = (pt, lp)
                            continue
                        if p == 0 and t == 1 and h in deferred_p0:
                            nc.tensor.matmul(
                                o_by_head[h][0][:], lhsT=lp,
                                rhs=vsb[:, VROW:2 * VROW],
                                start=True, stop=False)
                            _, lp0 = deferred_p0.pop(h)
                            nc.tensor.matmul(
                                o_by_head[h][0][:], lhsT=lp0,
                                rhs=vsb[:, 0:VROW],
                                start=False, stop=False)
                            continue
                        nc.tensor.matmul(
                            o_by_head[h][p][:],
                            lhsT=lp,
                            rhs=vsb[:, t * VROW:(t + 1) * VROW],
                            start=(t == 0),
                            stop=(t == NCOV[p] - 1),
                        )

            evq = []   # (h, p, en) awaiting their PE transposes

            def emit_evict_scale(i):
                h, grp = all_groups[i]
                for p in range(NSLOT):
                    if NCOV[p] - 1 != grp[-1]:
                        continue
                    o_tile = o_by_head[h][p]
                    r = rcpp.tile([128, 1], fp32, tag="r")
                    nc.vector.reciprocal(r[:], o_tile[:, HEAD_DIM:VROW])
                    en = enp.tile([128, HEAD_DIM], bf16, tag="en")
                    nc.scalar.activation(en[:], o_tile[:, 0:HEAD_DIM], Copy,
                                         scale=r[:])
                    evq.append((h, p, en))

            def emit_transposes():
                # XBAR DMA transposes: keeps the tensor engine and DVE out
                # of the eviction path entirely (SP + DMA are idle here).
                while evq:
                    h, p, en = evq.pop(0)
                    for hh in range(2):
                        nc.sync.dma_start_transpose(
                            out=enct[:, (2 * h + hh) * QBLK + p * 128:
                                     (2 * h + hh) * QBLK + (p + 1) * 128],
                            in_=en[:, hh * 128:(hh + 1) * 128])

            # wf prefetch: triggers fire on the idle SP queue during attention
            load_w(wf, 0)
            load_w(wf, 1)

            NG = len(all_groups)
            LAG = 3
            for i in range(NG):
                emit_s(i)
                if i >= LAG:
                    emit_o(i - LAG)
                    emit_evict_scale(i - LAG)   # recip/scale queue ahead...
                emit_mask(i)                    # ...of this group's mask-mul
                if i >= LAG:
                    emit_transposes()       # drain earlier evictions

            # Attention tail interleaved with the first final-proj stripes:
            # heads 0..8 (k=0..17) of stripes 0/1 accumulate while head 9's
            # last O/eviction chains drain, hiding their latency.
            fin_ps = {}

            def final_partial(m, kr):
                if m not in fin_ps:
                    fin_ps[m] = stp.tile([128, QBLK], fp32, tag="st",
                                         name=f"fps{m}")
                for k in kr:
                    nc.tensor.matmul(
                        fin_ps[m][:],
                        lhsT=wq_tiles[("wf", m)][:, k * 128:(k + 1) * 128],
                        rhs=enct[:, k * QBLK:(k + 1) * QBLK],
                        start=(k == 0),
                        stop=(k == NW - 1),
                    )

            emit_o(NG - 3)
            emit_evict_scale(NG - 3)
            emit_o(NG - 2)
            emit_evict_scale(NG - 2)
            final_partial(0, range(0, 18))
            emit_o(NG - 1)
            emit_evict_scale(NG - 1)
            final_partial(1, range(0, 18))
            emit_transposes()

            # ---- final projection: out^T = wf @ enc^T + bias ----
            for m in range(NW):
                if m + 2 < NW:
                    load_w(wf, m + 2)
                wf_m = wq_tiles.pop(("wf", m))
                if m in fin_ps:  # heads 0..8 already accumulated above
                    ps = fin_ps.pop(m)
                    for k in range(18, NW):
                        nc.tensor.matmul(
                            ps[:],
                            lhsT=wf_m[:, k * 128:(k + 1) * 128],
                            rhs=enct[:, k * QBLK:(k + 1) * QBLK],
                            start=(k == 0),
                            stop=(k == NW - 1),
                        )
                    osb = outp.tile([128, QBLK], fp32, tag="osb")
                    nc.vector.tensor_scalar_add(osb[:], ps[:], bia_s[:, m:m + 1])
                    nc.sync.dma_start(out=out[m], in_=osb[:])
                    continue
                ps = pp.tile([128, QBLK], fp32, tag="pp")
                if m < NW - 1:
                    for k in range(NW):
                        nc.tensor.matmul(
                            ps[:],
                            lhsT=wf_m[:, k * 128:(k + 1) * 128],
                            rhs=enct[:, k * QBLK:(k + 1) * QBLK],
                            start=(k == 0),
                            stop=(k == NW - 1),
                        )
                    osb = outp.tile([128, QBLK], fp32, tag="osb")
                    nc.vector.tensor_scalar_add(osb[:], ps[:], bia_s[:, m:m + 1])
                    nc.sync.dma_start(out=out[m], in_=osb[:])
                else:
                    # last stripe in column halves (separate PSUM tiles): the
                    # first half's bias/store drains under the second half
                    osb = outp.tile([128, QBLK], fp32, tag="osb")
                    pieces = [(0, 256), (256, 384), (384, 512)]
                    for j2, (c0, c1) in enumerate(pieces):
                        cw = c1 - c0
                        psj = ps if j2 == 0 else pp.tile(
                            [128, QBLK], fp32, tag="pp")
                        for k in range(NW):
                            nc.tensor.matmul(
                                psj[:, 0:cw],
                                lhsT=wf_m[:, k * 128:(k + 1) * 128],
                                rhs=enct[:, k * QBLK + c0:k * QBLK + c1],
                                start=(k == 0),
                                stop=(k == NW - 1),
                            )
                        nc.vector.tensor_scalar_add(
                            osb[:, c0:c1], psj[:, 0:cw], bia_s[:, m:m + 1])
                        nc.sync.dma_start(
                            out=out[m][:, c0:c1], in_=osb[:, c0:c1])

    if not nc.is_finalized():
        nc.finalize()  # bacc register allocation — required before walrus compile
    return nc


def get_nc():
    if "nc" not in _NC_CACHE:
        _NC_CACHE["nc"] = _build_nc()
    return _NC_CACHE["nc"]


def _chunk_of_slot(r, p):
    """Physical 128-token query chunk held by rank r's slot p."""
    return r + 12 - 4 * p


def _host_prepare(x, segment_pos, wq, wk, wv, w_final, b_final):
    """Build shared + per-core device input arrays."""
    x = np.asarray(x, dtype=np.float32)
    segment_pos = np.asarray(segment_pos)
    wq = np.asarray(wq, dtype=np.float32)
    wk = np.asarray(wk, dtype=np.float32)
    wv = np.asarray(wv, dtype=np.float32)
    w_final = np.asarray(w_final, dtype=np.float32)
    b_final = np.asarray(b_final, dtype=np.float32)

    def stripes_sq(w):  # [WIDTH, WIDTH] torch-Linear weight -> [20,128,WIDTH] w^T stripes
        wt = np.ascontiguousarray(w.T)
        return np.ascontiguousarray(
            wt.reshape(NW, 128, NW, 128).transpose(2, 1, 0, 3).reshape(NW, 128, WIDTH)
        ).astype(BF16)

    def packed_kv(w):  # [HEAD_DIM, WIDTH] -> [128, NW, HEAD_DIM] w^T stripes
        return w.T.reshape(NW, 128, HEAD_DIM).transpose(1, 0, 2)

    wk_p = packed_kv(wk)
    wv_p = packed_kv(wv)

    shared = {
        "wq": stripes_sq(wq),
        "wf": stripes_sq(w_final),
        "bia": np.ascontiguousarray(b_final.reshape(NW, 128).T).astype(np.float32),
    }

    inv_freq = (
        1.0 / MAX_WAVELENGTH ** (2.0 * np.arange(HEAD_DIM // 4, dtype=np.float32)
                                 / (HEAD_DIM // 2))
    ).astype(np.float32)

    def cossin(pos):
        ang = inv_freq[:, None] * pos[None, :].astype(np.float32)
        return (np.cos(ang).astype(BF16), np.sin(ang).astype(BF16))

    in_maps = []
    for c in range(8):
        b, r = c // 4, c % 4
        pos = segment_pos[b].astype(np.float32)
        seg = np.cumsum((segment_pos[b] == 0).astype(np.int64))

        qidx = np.concatenate(
            [np.arange(128) + 128 * _chunk_of_slot(r, p) for p in range(NSLOT)])
        kidx = np.arange(KVB) + KVB * r

        xqc = np.ascontiguousarray(x[b][qidx].T).astype(BF16).reshape(NW, 128, QBLK)
        xkv_p = x[b][kidx].T.reshape(NW, 128, KVB).transpose(1, 0, 2)
        kvw_c = np.empty((128, NW, 1024), dtype=BF16)
        kvw_c[:, :, 0:256] = wk_p
        kvw_c[:, :, 256:512] = wv_p
        kvw_c[:, :, 512:1024] = xkv_p
        kvw_c = kvw_c.reshape(128, NW * 1024)

        csq_, snq_ = cossin(pos[qidx])
        csk_, snk_ = cossin(pos[kidx])
        qa_ = np.vstack([csq_, snq_])
        qb_ = np.vstack([snq_, csq_])
        ka_ = np.vstack([csk_, snk_])
        kb_ = np.vstack([snk_, csk_])

        # packed masks: for key tile t, needed(t) slot blocks of [128k,128q]
        mask = np.zeros((128, MCOLS), dtype=BF16)
        for t in range(NTT):
            tk = np.arange(128) + 128 * t
            for p in range(_needed(t)):
                tq = qidx[p * 128:(p + 1) * 128]
                allow = (
                    (tk[:, None] <= tq[None, :])
                    & (tq[None, :] <= tk[:, None] + WINDOW)
                    & (seg[tk][:, None] == seg[tq][None, :])
                )
                mask[:, MOFF[t] + 128 * p: MOFF[t] + 128 * (p + 1)] = allow
        in_maps.append(dict(
            shared,
            xq=xqc,
            kvw=kvw_c,
            ka=ka_,
            kb=kb_,
            qa=qa_,
            qb=qb_,
            msk=mask,
        ))
    return in_maps


def _assemble(results):
    out = np.empty((B, T, WIDTH), dtype=np.float32)
    for c, res in enumerate(results):
        b, r = c // 4, c % 4
        o = np.asarray(res["out"], dtype=np.float32)  # [NW, 128, QBLK]
        for p in range(NSLOT):
            ch = _chunk_of_slot(r, p)
            out[b, 128 * ch:128 * (ch + 1), :] = (
                o[:, :, 128 * p:128 * (p + 1)].transpose(2, 0, 1).reshape(128, WIDTH)
            )
    return out


def kernel(x, segment_pos, wq, wk, wv, w_final, b_final):
    from concourse.bass_utils import run_bass_kernel_spmd

    nc = get_nc()
    in_maps = _host_prepare(x, segment_pos, wq, wk, wv, w_final, b_final)
    res = run_bass_kernel_spmd(nc, in_maps, list(range(8)))
    return _assemble(res.results)
